# revision 8
# baseline (speedup 1.0000x reference)
"""CIF token bridge kernel for Trainium2 (8 NeuronCores, data-parallel).

Per batch (B=16, H=1024, T=2048, K=192):
  conv(k=3, SAME) over time -> silu -> proj -> sigmoid -> masked alpha
  -> raw_mass / rescale -> cumsum centers -> triangular assignment
  -> normalized einsum against raw features.

Sharding: batch dim across 8 cores (2 batches/core); all params replicated.
Matmuls run in float32r (fast fp32 mode, ~11 mantissa bits); the CIF
cumsum runs in true fp32 via a two-level scan (128-row prefix matmul +
16-column scalar scan) to keep center drift ~1e-5.
"""
import os
import sys
import types

import numpy as np


def _install_ntff_hook():
    """antenv.axon_hooks is absent from this image; inject it and register
    the ctypes NTFF hook so trace=True yields HW exec times."""
    if "antenv.axon_hooks" in sys.modules:
        return
    mod = types.ModuleType("antenv.axon_hooks")
    mod._hook = None
    mod.set_axon_ntff_profile_hook = lambda h: setattr(mod, "_hook", h)
    mod.get_axon_ntff_profile_hook = lambda: mod._hook
    sys.modules["antenv.axon_hooks"] = mod
    try:
        import antenv

        antenv.axon_hooks = mod
    except ImportError:
        pass
    try:
        from trn_agent_boot.trn_boot import _ntff_profile_via_ctypes

        hook = _ntff_profile_via_ctypes("/opt/axon/libaxon_pjrt.so")
        if hook is not None:
            mod._hook = hook
    except Exception:
        pass


_install_ntff_hook()

import concourse.bass as bass  # noqa: E402
import concourse.tile as tile  # noqa: E402
from concourse import bacc, mybir  # noqa: E402
from concourse.bass_utils import run_bass_kernel_spmd  # noqa: E402

N_CORES = 8
B, H, T = 16, 1024, 2048
K = 192
MAX_TOK = 192
BPC = B // N_CORES  # batches per core
NT = T // 512       # 512-wide t tiles per batch
NH = H // 128       # 128-row chunks of H
NC16 = T // 128     # 128-wide t chunks per batch

f32 = mybir.dt.float32
f32r = mybir.dt.float32r
AF = mybir.ActivationFunctionType
ALU = mybir.AluOpType
AX = mybir.AxisListType

LAST_EXEC_NS = None
LAST_RESULTS = None


def _build_program():
    nc = bacc.Bacc(trn_type="TRN2", target_bir_lowering=False, debug=False,
                   num_devices=1)

    encp = nc.dram_tensor("encp", [BPC, H, T + 2], f32r, kind="ExternalInput").ap()
    encT = nc.dram_tensor("encT", [BPC, T, H], f32r, kind="ExternalInput").ap()
    wT = nc.dram_tensor("wT", [3, H, H], f32r, kind="ExternalInput").ap()
    convb = nc.dram_tensor("convb", [128, NH], f32, kind="ExternalInput").ap()
    projw = nc.dram_tensor("projw", [128, NH], f32r, kind="ExternalInput").ap()
    projb = nc.dram_tensor("projb", [1, 1], f32, kind="ExternalInput").ap()
    tokc = nc.dram_tensor("tokc", [128, K], f32, kind="ExternalInput").ap()
    tri = nc.dram_tensor("tri", [128, 128], f32, kind="ExternalInput").ap()
    ones2 = nc.dram_tensor("ones2", [128, 2], f32r, kind="ExternalInput").ap()
    tmask = nc.dram_tensor("tmask", [BPC, T], f32, kind="ExternalInput").ap()
    kmask = nc.dram_tensor("kmask", [BPC, K], f32, kind="ExternalInput").ap()
    toklen = nc.dram_tensor("toklen", [BPC], f32, kind="ExternalInput").ap()

    acoustic = nc.dram_tensor("acoustic", [BPC, K, H], f32,
                              kind="ExternalOutput").ap()
    rm_out = nc.dram_tensor("raw_mass", [BPC], f32, kind="ExternalOutput").ap()

    with tile.TileContext(nc) as tc:
        with (
            tc.tile_pool(name="wp", bufs=192) as wp,
            tc.tile_pool(name="cst", bufs=1) as cst,
            tc.tile_pool(name="encs", bufs=12) as encs,
            tc.tile_pool(name="sp", bufs=3) as sp,
            tc.tile_pool(name="rows", bufs=2) as rows,
            tc.tile_pool(name="pm", bufs=2) as pm,
            tc.tile_pool(name="asgp", bufs=17) as asgp,
            tc.tile_pool(name="etp", bufs=4) as etp,
            tc.tile_pool(name="otp", bufs=2) as otp,
            tc.tile_pool(name="cps", bufs=2, space="PSUM") as cpsp,
            tc.tile_pool(name="pps", bufs=1, space="PSUM") as ppsp,
            tc.tile_pool(name="xps", bufs=1, space="PSUM") as xpsp,
            tc.tile_pool(name="e0ps", bufs=2, space="PSUM") as e0psp,
            tc.tile_pool(name="e1ps", bufs=2, space="PSUM") as e1psp,
        ):
            # ---- constants resident in SBUF ----
            # weight chunks [128i, 128o] DMA'd lazily right before first use
            # so the first conv tile isn't queued behind 12.6 MB of weights
            w_sb = {}

            def w_chunk(k, i, o):
                key = (k, i, o)
                if key not in w_sb:
                    wt = wp.tile([128, 128], f32r, tag="w", name=f"w{k}_{i}_{o}")
                    nc.sync.dma_start(
                        wt[:], wT[k, i * 128:(i + 1) * 128,
                                  o * 128:(o + 1) * 128])
                    w_sb[key] = wt
                return w_sb[key]
            convb_sb = cst.tile([128, NH], f32)
            nc.sync.dma_start(convb_sb[:], convb[:])
            projw_sb = cst.tile([128, NH], f32r)
            nc.sync.dma_start(projw_sb[:], projw[:])
            projb_sb = cst.tile([1, 1], f32)
            nc.sync.dma_start(projb_sb[:], projb[:])
            tokc_sb = cst.tile([128, K], f32)
            nc.sync.dma_start(tokc_sb[:], tokc[:])
            tri_sb = cst.tile([128, 128], f32)
            nc.sync.dma_start(tri_sb[:], tri[:])
            ones_sb = cst.tile([128, 2], f32r)
            nc.sync.dma_start(ones_sb[:], ones2[:])

            alpha_rows = []

            # ---- phase A: conv -> silu -> proj -> sigmoid (both batches) ----
            for b in range(BPC):
                alpha_row = rows.tile([1, T], f32, tag="alpha")
                alpha_rows.append(alpha_row)
                for t in range(NT):
                    enc_t = []
                    for i in range(NH):
                        et = encs.tile([128, 514], f32r, tag="enc")
                        nc.sync.dma_start(
                            et[:], encp[b, i * 128:(i + 1) * 128,
                                        t * 512:t * 512 + 514])
                        enc_t.append(et)
                    pps = ppsp.tile([1, 512], f32, tag="proj")
                    for o in range(NH):
                        cps = cpsp.tile([128, 512], f32, tag="conv")
                        for k in range(3):
                            for i in range(NH):
                                nc.tensor.matmul(
                                    cps[:],
                                    w_chunk(k, i, o)[:],
                                    enc_t[i][:, k:k + 512],
                                    start=(k == 0 and i == 0),
                                    stop=(k == 2 and i == NH - 1))
                        s_t = sp.tile([128, 512], f32r, tag="s")
                        nc.scalar.activation(s_t[:], cps[:], AF.Silu,
                                             bias=convb_sb[:, o:o + 1],
                                             scale=1.0)
                        nc.tensor.matmul(pps[:], projw_sb[:, o:o + 1], s_t[:],
                                         start=(o == 0), stop=(o == NH - 1))
                    sig = sp.tile([1, 512], f32, tag="sig", bufs=2)
                    nc.scalar.activation(sig[:], pps[:], AF.Sigmoid,
                                         bias=projb_sb[0:1, 0:1], scale=1.0)
                    tmseg = sp.tile([1, 512], f32, tag="tmseg", bufs=3)
                    nc.sync.dma_start(tmseg[:],
                                      tmask[b, t * 512:(t + 1) * 512])
                    nc.vector.tensor_mul(alpha_row[0:1, t * 512:(t + 1) * 512],
                                         sig[:], tmseg[:])

            # ---- phase B: raw mass, rescale, two-level cumsum, centers ----
            sa_pms, cent_negs, sa_negs = [], [], []
            for b in range(BPC):
                alpha_row = alpha_rows[b]
                rm = pm.tile([1, 1], f32, tag="rm")
                nc.vector.tensor_reduce(rm[:], alpha_row[:], axis=AX.X, op=ALU.add)
                nc.sync.dma_start(rm_out[b:b + 1], rm[:])
                rmc = pm.tile([1, 1], f32, tag="rmc")
                nc.vector.tensor_scalar_max(rmc[:], rm[:], 1e-6)
                rmi = pm.tile([1, 1], f32, tag="rmi")
                nc.vector.reciprocal(rmi[:], rmc[:])
                tl_sb = pm.tile([1, 1], f32, tag="tl")
                nc.sync.dma_start(tl_sb[:], toklen[b:b + 1])
                scl = pm.tile([1, 1], f32, tag="scl")
                nc.vector.tensor_mul(scl[:], rmi[:], tl_sb[:])

                sa_row = alpha_row
                nc.vector.tensor_scalar(sa_row[:], alpha_row[:],
                                        scl[0:1, 0:1], None, op0=ALU.mult)
                sa_pm = pm.tile([128, NC16], f32, tag="sa_pm")
                for c in range(NC16):
                    nc.sync.dma_start(sa_pm[:, c:c + 1],
                                      sa_row[0:1, c * 128:(c + 1) * 128])
                pref = xpsp.tile([128, 2 * NC16], f32, tag="small")
                nc.tensor.matmul(pref[:, 0:NC16], tri_sb[:], sa_pm[:],
                                 start=True, stop=True)
                # column totals into partition 0 (tri[:,127] is all ones)
                nc.tensor.matmul(pref[0:1, NC16:2 * NC16],
                                 tri_sb[:, 127:128], sa_pm[:],
                                 start=True, stop=True)
                colsum = pm.tile([1, NC16], f32, tag="colsum")
                nc.vector.tensor_copy(colsum[:], pref[0:1, NC16:2 * NC16])
                incl = pm.tile([1, NC16], f32, tag="incl")
                nc.vector.tensor_tensor_scan(incl[:], colsum[:], colsum[:],
                                             0.0, op0=ALU.add, op1=ALU.bypass)
                offs = pm.tile([1, NC16], f32, tag="offs")
                nc.vector.scalar_tensor_tensor(offs[:], colsum[:], -1.0,
                                               incl[:], op0=ALU.mult,
                                               op1=ALU.add)
                offs_bc = pm.tile([128, NC16], f32, tag="offs_bc")
                nc.gpsimd.partition_broadcast(offs_bc[:], offs[0:1, :])
                cmb = pm.tile([128, NC16], f32, tag="cmb")
                nc.vector.scalar_tensor_tensor(cmb[:], sa_pm[:], -0.5,
                                               pref[:, 0:NC16], op0=ALU.mult,
                                               op1=ALU.add)
                cent_pm = pm.tile([128, NC16], f32, tag="cent_pm")
                nc.vector.tensor_add(cent_pm[:], cmb[:], offs_bc[:])
                cent_neg = pm.tile([128, NC16], f32, tag="cent_neg")
                nc.vector.tensor_scalar_mul(cent_neg[:], cent_pm[:], -1.0)
                sa_neg = pm.tile([128, NC16], f32, tag="sa_neg")
                nc.vector.tensor_scalar_mul(sa_neg[:], sa_pm[:], -1.0)
                sa_pms.append(sa_pm)
                cent_negs.append(cent_neg)
                sa_negs.append(sa_neg)

            # ---- phases C+D: assignment build + einsum (per batch) ----
            for b in range(BPC):
                sa_pm, cent_neg, sa_neg = sa_pms[b], cent_negs[b], sa_negs[b]
                asg = []
                for c in range(NC16):
                    td = asgp.tile([128, K], f32, tag="td", bufs=3)
                    nc.scalar.activation(td[:], tokc_sb[:], AF.Abs,
                                         bias=cent_neg[:, c:c + 1], scale=1.0)
                    at = asgp.tile([128, K], f32r, tag="asg")
                    nc.scalar.activation(at[:], td[:], AF.Relu,
                                         bias=sa_pm[:, c:c + 1],
                                         scale=sa_neg[:, c:c + 1])
                    asg.append(at)

                kms = []
                for kc, m in ((0, 128), (1, 64)):
                    km = pm.tile([m, 1], f32, tag=f"km{kc}")
                    nc.sync.dma_start(km[:], kmask[b, kc * 128:kc * 128 + m])
                    kms.append(km)
                scs = []
                for kc, m in ((0, 128), (1, 64)):
                    dn = xpsp.tile([128, 2 * NC16], f32, tag="small")
                    for c in range(NC16):
                        nc.tensor.matmul(dn[:m, 0:2],
                                         asg[c][:, kc * 128:kc * 128 + m],
                                         ones_sb[:], start=(c == 0),
                                         stop=(c == NC16 - 1))
                    sc = pm.tile([m, 1], f32, tag=f"sc{kc}")
                    nc.vector.tensor_scalar_max(sc[:], dn[:m, 0:1], 1e-6)
                    sci = pm.tile([m, 1], f32, tag=f"sci{kc}")
                    nc.vector.reciprocal(sci[:], sc[:])
                    scm = pm.tile([m, 1], f32, tag=f"scm{kc}")
                    nc.vector.tensor_mul(scm[:], sci[:], kms[kc][:])
                    scs.append(scm)

                for hh in range(2):
                    ps0 = e0psp.tile([128, 512], f32, tag="e0")
                    ps1 = e1psp.tile([64, 512], f32, tag="e1")
                    pss = [ps0, ps1]
                    for c in range(NC16):
                        et = etp.tile([128, 512], f32r, tag="et")
                        nc.sync.dma_start(
                            et[:], encT[b, c * 128:(c + 1) * 128,
                                        hh * 512:(hh + 1) * 512])
                        nc.tensor.matmul(pss[0][:], asg[c][:, 0:128], et[:],
                                         start=(c == 0), stop=(c == NC16 - 1))
                        nc.tensor.matmul(pss[1][:], asg[c][:, 128:192], et[:],
                                         start=(c == 0), stop=(c == NC16 - 1))
                    for kc, m in ((0, 128), (1, 64)):
                        ot = otp.tile([m, 512], f32, tag=f"ot{kc}")
                        nc.vector.tensor_scalar(ot[:], pss[kc][:],
                                                scs[kc][:, 0:1], None,
                                                op0=ALU.mult)
                        nc.sync.dma_start(
                            acoustic[b, kc * 128:kc * 128 + m,
                                     hh * 512:(hh + 1) * 512], ot[:])
    nc.compile()
    return nc


_PROGRAM = None


def _get_program():
    global _PROGRAM
    if _PROGRAM is None:
        _PROGRAM = _build_program()
    return _PROGRAM


def kernel(encoder_outputs, encoder_lengths, target_token_lengths,
           conv_w, conv_b, proj_w, proj_b):
    global LAST_EXEC_NS, LAST_RESULTS
    enc = np.asarray(encoder_outputs, dtype=np.float32)
    enc_len = np.asarray(encoder_lengths, dtype=np.int32)
    tgt_len = np.asarray(target_token_lengths, dtype=np.int32)
    conv_w = np.asarray(conv_w, dtype=np.float32)
    conv_b = np.asarray(conv_b, dtype=np.float32)
    proj_w = np.asarray(proj_w, dtype=np.float32)
    proj_b = np.asarray(proj_b, dtype=np.float32)

    enc_pad = np.zeros((B, H, T + 2), dtype=np.float32)
    enc_pad[:, :, 1:-1] = enc
    encT = np.ascontiguousarray(enc.transpose(0, 2, 1))
    wT = np.ascontiguousarray(conv_w.transpose(2, 1, 0))
    convb_r = np.ascontiguousarray(conv_b.reshape(NH, 128).T)
    projw_r = np.ascontiguousarray(proj_w.reshape(NH, 128).T)
    projb_v = proj_b.reshape(1, 1)
    tokc = np.ascontiguousarray(
        np.broadcast_to(np.arange(K, dtype=np.float32) + 0.5, (128, K)))
    tri = np.triu(np.ones((128, 128), dtype=np.float32))
    ones2 = np.ones((128, 2), dtype=np.float32)

    tok_len = np.clip(tgt_len, 1, MAX_TOK).astype(np.int32)
    toklen_f = tok_len.astype(np.float32)
    tmask = (np.arange(T)[None, :] < enc_len[:, None]).astype(np.float32)
    kmask = (np.arange(K)[None, :] < tok_len[:, None]).astype(np.float32)

    in_maps = []
    for c in range(N_CORES):
        s = slice(c * BPC, (c + 1) * BPC)
        in_maps.append({
            "encp": enc_pad[s], "encT": encT[s], "wT": wT,
            "convb": convb_r, "projw": projw_r, "projb": projb_v,
            "tokc": tokc, "tri": tri, "ones2": ones2,
            "tmask": tmask[s], "kmask": kmask[s], "toklen": toklen_f[s],
        })

    nc = _get_program()
    trace = bool(os.environ.get("KERNEL_TRACE"))
    res = run_bass_kernel_spmd(nc, in_maps, list(range(N_CORES)), trace=trace)
    LAST_EXEC_NS = res.exec_time_ns
    LAST_RESULTS = res

    acoustic = np.concatenate([res.results[c]["acoustic"]
                               for c in range(N_CORES)], axis=0)
    raw_mass = np.concatenate([res.results[c]["raw_mass"]
                               for c in range(N_CORES)], axis=0)
    quantity_loss = np.float32(np.mean(np.abs(raw_mass - toklen_f)))
    return acoustic, tok_len, quantity_loss


# revision 11
# speedup vs baseline: 1.1449x; 1.1449x over previous
"""CIF token bridge kernel for Trainium2 (8 NeuronCores, data-parallel).

Per batch (B=16, H=1024, T=2048, K=192):
  conv(k=3, SAME) over time -> silu -> proj -> sigmoid -> masked alpha
  -> raw_mass / rescale -> cumsum centers -> triangular assignment
  -> normalized einsum against raw features.

Sharding: batch dim across 8 cores (2 batches/core); all params replicated.
Matmuls run in float32r (fast fp32 mode, ~11 mantissa bits); the CIF
cumsum runs in true fp32 via a two-level scan (free-dim hardware scan +
a strict-triangular fp32 matmul for row offsets) to keep center drift
~1e-5.

Time layout for the assignment/einsum stages is "p-major": tile column c,
partition p holds t = p*16 + c. The host pre-permutes the feature matrix
(encT) to match, so the einsum contraction sees a consistent t ordering.
"""
import os
import sys
import types

import numpy as np


def _install_ntff_hook():
    """antenv.axon_hooks is absent from this image; inject it and register
    the ctypes NTFF hook so trace=True yields HW exec times."""
    if "antenv.axon_hooks" in sys.modules:
        return
    mod = types.ModuleType("antenv.axon_hooks")
    mod._hook = None
    mod.set_axon_ntff_profile_hook = lambda h: setattr(mod, "_hook", h)
    mod.get_axon_ntff_profile_hook = lambda: mod._hook
    sys.modules["antenv.axon_hooks"] = mod
    try:
        import antenv

        antenv.axon_hooks = mod
    except ImportError:
        pass
    try:
        from trn_agent_boot.trn_boot import _ntff_profile_via_ctypes

        hook = _ntff_profile_via_ctypes("/opt/axon/libaxon_pjrt.so")
        if hook is not None:
            mod._hook = hook
    except Exception:
        pass


_install_ntff_hook()

import concourse.bass as bass  # noqa: E402
import concourse.tile as tile  # noqa: E402
from concourse import bacc, mybir  # noqa: E402
from concourse.bass_utils import run_bass_kernel_spmd  # noqa: E402

N_CORES = 8
B, H, T = 16, 1024, 2048
K = 192
MAX_TOK = 192
BPC = B // N_CORES  # batches per core
NT = T // 512       # 512-wide t tiles per batch
NH = H // 128       # 128-row chunks of H
NC16 = T // 128     # t chunks per batch (tile columns in p-major layout)

f32 = mybir.dt.float32
f32r = mybir.dt.float32r
AF = mybir.ActivationFunctionType
ALU = mybir.AluOpType
AX = mybir.AxisListType

LAST_EXEC_NS = None
LAST_RESULTS = None


def _build_program():
    nc = bacc.Bacc(trn_type="TRN2", target_bir_lowering=False, debug=False,
                   num_devices=1)

    encp = nc.dram_tensor("encp", [BPC, H, T + 2], f32r, kind="ExternalInput").ap()
    # encTp[b, c, p, :] = feats[b, p*16 + c, :]
    encTp = nc.dram_tensor("encTp", [BPC, NC16, 128, H], f32r,
                           kind="ExternalInput").ap()
    wT = nc.dram_tensor("wT", [3, H, H], f32r, kind="ExternalInput").ap()
    convb = nc.dram_tensor("convb", [128, NH], f32, kind="ExternalInput").ap()
    projw = nc.dram_tensor("projw", [128, NH], f32r, kind="ExternalInput").ap()
    projb = nc.dram_tensor("projb", [1, 1], f32, kind="ExternalInput").ap()
    tokc = nc.dram_tensor("tokc", [128, K], f32, kind="ExternalInput").ap()
    # tri_s[p, m] = 1.0 if p < m else 0.0 (strict lower prefix as lhsT)
    tri_s = nc.dram_tensor("tri_s", [128, 128], f32, kind="ExternalInput").ap()
    ones2 = nc.dram_tensor("ones2", [128, 2], f32r, kind="ExternalInput").ap()
    tmask = nc.dram_tensor("tmask", [BPC, T], f32, kind="ExternalInput").ap()
    kmask = nc.dram_tensor("kmask", [BPC, K], f32, kind="ExternalInput").ap()
    toklen = nc.dram_tensor("toklen", [BPC], f32, kind="ExternalInput").ap()

    acoustic = nc.dram_tensor("acoustic", [BPC, K, H], f32,
                              kind="ExternalOutput").ap()
    rm_out = nc.dram_tensor("raw_mass", [BPC], f32, kind="ExternalOutput").ap()

    with tile.TileContext(nc) as tc:
        with (
            tc.tile_pool(name="wp", bufs=24) as wp,
            tc.tile_pool(name="cst", bufs=1) as cst,
            tc.tile_pool(name="encs", bufs=12) as encs,
            tc.tile_pool(name="sp", bufs=3) as sp,
            tc.tile_pool(name="rows", bufs=2) as rows,
            tc.tile_pool(name="pm", bufs=2) as pm,
            tc.tile_pool(name="asgp", bufs=17) as asgp,
            tc.tile_pool(name="etp", bufs=4) as etp,
            tc.tile_pool(name="otp", bufs=2) as otp,
            tc.tile_pool(name="cps", bufs=2, space="PSUM") as cpsp,
            tc.tile_pool(name="pps", bufs=1, space="PSUM") as ppsp,
            tc.tile_pool(name="xps", bufs=1, space="PSUM") as xpsp,
            tc.tile_pool(name="e0ps", bufs=2, space="PSUM") as e0psp,
            tc.tile_pool(name="e1ps", bufs=2, space="PSUM") as e1psp,
        ):
            # ---- small constants first (cheap, unblock early compute) ----
            convb_sb = cst.tile([128, NH], f32)
            nc.sync.dma_start(convb_sb[:], convb[:])
            projw_sb = cst.tile([128, NH], f32r)
            nc.sync.dma_start(projw_sb[:], projw[:])
            projb_sb = cst.tile([1, 1], f32)
            nc.sync.dma_start(projb_sb[:], projb[:])
            tokc_sb = cst.tile([128, K], f32)
            nc.sync.dma_start(tokc_sb[:], tokc[:])
            tri_sb = cst.tile([128, 128], f32)
            nc.sync.dma_start(tri_sb[:], tri_s[:])
            ones_sb = cst.tile([128, 2], f32r)
            nc.sync.dma_start(ones_sb[:], ones2[:])

            # ---- first batch\'s first enc slab BEFORE the 12.6MB of weights
            # so the PE isn\'t starved at kernel start ----
            enc_slabs = {}

            def load_slab(b, t):
                tiles = []
                for i in range(NH):
                    et = encs.tile([128, 514], f32r, tag="enc",
                                   name=f"enc{b}_{t}_{i}")
                    nc.sync.dma_start(
                        et[:], encp[b, i * 128:(i + 1) * 128,
                                    t * 512:t * 512 + 514])
                    tiles.append(et)
                enc_slabs[(b, t)] = tiles

            load_slab(0, 0)

            w_sb = []
            for k in range(3):
                for i in range(NH):
                    wt = wp.tile([128, H], f32r, tag="w", name=f"w{k}_{i}")
                    nc.sync.dma_start(wt[:], wT[k, i * 128:(i + 1) * 128, :])
                    w_sb.append(wt)

            alpha_rows = {}

            def conv_tile(b, t):
                """conv -> silu -> proj -> sigmoid*mask for one 512 tile."""
                if (b, t) not in enc_slabs:
                    load_slab(b, t)
                enc_t = enc_slabs.pop((b, t))
                if t == 0:
                    alpha_rows[b] = rows.tile([1, T], f32, tag="alpha",
                                              name=f"alpha{b}")
                alpha_row = alpha_rows[b]
                pps = ppsp.tile([1, 512], f32, tag="proj", name=f"pj{b}_{t}")
                for o in range(NH):
                    cps = cpsp.tile([128, 512], f32, tag="conv",
                                    name=f"cv{b}_{t}_{o}")
                    for k in range(3):
                        for i in range(NH):
                            nc.tensor.matmul(
                                cps[:],
                                w_sb[k * NH + i][:, o * 128:(o + 1) * 128],
                                enc_t[i][:, k:k + 512],
                                start=(k == 0 and i == 0),
                                stop=(k == 2 and i == NH - 1))
                    s_t = sp.tile([128, 512], f32r, tag="s",
                                  name=f"s{b}_{t}_{o}")
                    nc.scalar.activation(s_t[:], cps[:], AF.Silu,
                                         bias=convb_sb[:, o:o + 1], scale=1.0)
                    nc.tensor.matmul(pps[:], projw_sb[:, o:o + 1], s_t[:],
                                     start=(o == 0), stop=(o == NH - 1))
                sig = sp.tile([1, 512], f32, tag="sig", bufs=2,
                              name=f"sig{b}_{t}")
                nc.scalar.activation(sig[:], pps[:], AF.Sigmoid,
                                     bias=projb_sb[0:1, 0:1], scale=1.0)
                tmseg = sp.tile([1, 512], f32, tag="tmseg", bufs=3,
                                name=f"tms{b}_{t}")
                nc.sync.dma_start(tmseg[:], tmask[b, t * 512:(t + 1) * 512])
                nc.vector.tensor_mul(alpha_row[0:1, t * 512:(t + 1) * 512],
                                     sig[:], tmseg[:])

            def phase_b(b):
                """raw mass, rescale, two-level cumsum, centers (p-major)."""
                alpha_row = alpha_rows[b]
                rm = pm.tile([1, 1], f32, tag="rm", name=f"rm{b}")
                nc.vector.tensor_reduce(rm[:], alpha_row[:], axis=AX.X,
                                        op=ALU.add)
                nc.sync.dma_start(rm_out[b:b + 1], rm[:])
                rmc = pm.tile([1, 1], f32, tag="rmc", name=f"rmc{b}")
                nc.vector.tensor_scalar_max(rmc[:], rm[:], 1e-6)
                rmi = pm.tile([1, 1], f32, tag="rmi", name=f"rmi{b}")
                nc.vector.reciprocal(rmi[:], rmc[:])
                tl_sb = pm.tile([1, 1], f32, tag="tl", name=f"tl{b}")
                nc.sync.dma_start(tl_sb[:], toklen[b:b + 1])
                scl = pm.tile([1, 1], f32, tag="scl", name=f"scl{b}")
                nc.vector.tensor_mul(scl[:], rmi[:], tl_sb[:])

                sa_row = alpha_row
                nc.vector.tensor_scalar(sa_row[:], alpha_row[:],
                                        scl[0:1, 0:1], None, op0=ALU.mult)
                # p-major reshape: sa_pm[p, c] = sa[p*16 + c] (single DMA)
                sa_pm = pm.tile([128, NC16], f32, tag="sa_pm", name=f"sapm{b}")
                nc.sync.dma_start(sa_pm[:], sa_row[:])
                # in-row inclusive prefix (free-dim hardware scan)
                pref = pm.tile([128, NC16], f32, tag="pref", name=f"pref{b}")
                nc.vector.tensor_tensor_scan(pref[:], sa_pm[:], sa_pm[:],
                                             0.0, op0=ALU.add, op1=ALU.bypass)
                # row offsets: offs[m] = sum_{p<m} rowtot[p] (strict tri fp32)
                offs = xpsp.tile([128, 2], f32, tag="small", name=f"offs{b}")
                nc.tensor.matmul(offs[:, 0:2], tri_sb[:],
                                 pref[:, NC16 - 2:NC16], start=True, stop=True)
                # centers = pref + offs - 0.5*sa ; negated copies for ACT bias
                cmb = pm.tile([128, NC16], f32, tag="cmb", name=f"cmb{b}")
                nc.vector.scalar_tensor_tensor(cmb[:], sa_pm[:], -0.5,
                                               pref[:], op0=ALU.mult,
                                               op1=ALU.add)
                cent_neg = pm.tile([128, NC16], f32, tag="cent_neg",
                                   name=f"cn{b}")
                nc.vector.tensor_scalar(cent_neg[:], cmb[:], offs[:, 1:2],
                                        -1.0, op0=ALU.add, op1=ALU.mult)
                sa_neg = pm.tile([128, NC16], f32, tag="sa_neg", name=f"sn{b}")
                nc.vector.tensor_scalar_mul(sa_neg[:], sa_pm[:], -1.0)
                return sa_pm, cent_neg, sa_neg

            asg_tiles = {}

            def phase_c(b, sa_pm, cent_neg, sa_neg):
                """assignment tiles: relu(1-|tokc-cent|)*sa, f32r."""
                asg = []
                for c in range(NC16):
                    td = asgp.tile([128, K], f32, tag="td", bufs=3,
                                   name=f"td{b}_{c}")
                    nc.scalar.activation(td[:], tokc_sb[:], AF.Abs,
                                         bias=cent_neg[:, c:c + 1], scale=1.0)
                    at = asgp.tile([128, K], f32r, tag="asg",
                                   name=f"at{b}_{c}")
                    nc.scalar.activation(at[:], td[:], AF.Relu,
                                         bias=sa_pm[:, c:c + 1],
                                         scale=sa_neg[:, c:c + 1])
                    asg.append(at)
                asg_tiles[b] = asg

            scs_tiles = {}

            def phase_d_denom(b):
                asg = asg_tiles[b]
                kms = []
                for kc, m in ((0, 128), (1, 64)):
                    km = pm.tile([m, 1], f32, tag=f"km{kc}",
                                 name=f"km{b}_{kc}")
                    nc.sync.dma_start(km[:], kmask[b, kc * 128:kc * 128 + m])
                    kms.append(km)
                scs = []
                for kc, m in ((0, 128), (1, 64)):
                    dn = xpsp.tile([128, 2], f32, tag="small",
                                   name=f"dn{b}_{kc}")
                    for c in range(NC16):
                        nc.tensor.matmul(dn[:m, 0:2],
                                         asg[c][:, kc * 128:kc * 128 + m],
                                         ones_sb[:], start=(c == 0),
                                         stop=(c == NC16 - 1))
                    sc = pm.tile([m, 1], f32, tag=f"sc{kc}",
                                 name=f"sc{b}_{kc}")
                    nc.vector.tensor_scalar_max(sc[:], dn[:m, 0:1], 1e-6)
                    sci = pm.tile([m, 1], f32, tag=f"sci{kc}",
                                  name=f"sci{b}_{kc}")
                    nc.vector.reciprocal(sci[:], sc[:])
                    scm = pm.tile([m, 1], f32, tag=f"scm{kc}",
                                  name=f"scm{b}_{kc}")
                    nc.vector.tensor_mul(scm[:], sci[:], kms[kc][:])
                    scs.append(scm)
                scs_tiles[b] = scs

            def phase_d_einsum(b, hh):
                asg = asg_tiles[b]
                scs = scs_tiles[b]
                ps0 = e0psp.tile([128, 512], f32, tag="e0",
                                 name=f"e0{b}_{hh}")
                ps1 = e1psp.tile([64, 512], f32, tag="e1", name=f"e1{b}_{hh}")
                pss = [ps0, ps1]
                for c in range(NC16):
                    et = etp.tile([128, 512], f32r, tag="et",
                                  name=f"et{b}_{hh}_{c}")
                    nc.sync.dma_start(
                        et[:], encTp[b, c, :, hh * 512:(hh + 1) * 512])
                    nc.tensor.matmul(pss[0][:], asg[c][:, 0:128], et[:],
                                     start=(c == 0), stop=(c == NC16 - 1))
                    nc.tensor.matmul(pss[1][:], asg[c][:, 128:192], et[:],
                                     start=(c == 0), stop=(c == NC16 - 1))
                for kc, m in ((0, 128), (1, 64)):
                    ot = otp.tile([m, 512], f32, tag=f"ot{kc}",
                                  name=f"ot{b}_{hh}_{kc}")
                    nc.vector.tensor_scalar(ot[:], pss[kc][:],
                                            scs[kc][:, 0:1], None,
                                            op0=ALU.mult)
                    nc.sync.dma_start(
                        acoustic[b, kc * 128:kc * 128 + m,
                                 hh * 512:(hh + 1) * 512], ot[:])

            # ---- emission schedule ----
            for t in range(NT):
                conv_tile(0, t)
            sa0 = phase_b(0)
            phase_c(0, *sa0)
            # interleave einsum(b0) with conv(b1) to spread encT DMA load
            conv_tile(1, 0)
            phase_d_denom(0)
            conv_tile(1, 1)
            phase_d_einsum(0, 0)
            conv_tile(1, 2)
            phase_d_einsum(0, 1)
            conv_tile(1, 3)
            sa1 = phase_b(1)
            phase_c(1, *sa1)
            phase_d_denom(1)
            phase_d_einsum(1, 0)
            phase_d_einsum(1, 1)
    nc.compile()
    return nc


_PROGRAM = None


def _get_program():
    global _PROGRAM
    if _PROGRAM is None:
        _PROGRAM = _build_program()
    return _PROGRAM


def kernel(encoder_outputs, encoder_lengths, target_token_lengths,
           conv_w, conv_b, proj_w, proj_b):
    global LAST_EXEC_NS, LAST_RESULTS
    enc = np.asarray(encoder_outputs, dtype=np.float32)
    enc_len = np.asarray(encoder_lengths, dtype=np.int32)
    tgt_len = np.asarray(target_token_lengths, dtype=np.int32)
    conv_w = np.asarray(conv_w, dtype=np.float32)
    conv_b = np.asarray(conv_b, dtype=np.float32)
    proj_w = np.asarray(proj_w, dtype=np.float32)
    proj_b = np.asarray(proj_b, dtype=np.float32)

    enc_pad = np.zeros((B, H, T + 2), dtype=np.float32)
    enc_pad[:, :, 1:-1] = enc
    # p-major permuted feats: encTp[b, c, p, :] = enc[b, :, p*16 + c].T
    encTp = np.ascontiguousarray(
        enc.transpose(0, 2, 1).reshape(B, 128, NC16, H).transpose(0, 2, 1, 3))
    wT = np.ascontiguousarray(conv_w.transpose(2, 1, 0))
    convb_r = np.ascontiguousarray(conv_b.reshape(NH, 128).T)
    projw_r = np.ascontiguousarray(proj_w.reshape(NH, 128).T)
    projb_v = proj_b.reshape(1, 1)
    tokc = np.ascontiguousarray(
        np.broadcast_to(np.arange(K, dtype=np.float32) + 0.5, (128, K)))
    tri_strict = np.triu(np.ones((128, 128), dtype=np.float32), 1)
    ones2 = np.ones((128, 2), dtype=np.float32)

    tok_len = np.clip(tgt_len, 1, MAX_TOK).astype(np.int32)
    toklen_f = tok_len.astype(np.float32)
    tmask = (np.arange(T)[None, :] < enc_len[:, None]).astype(np.float32)
    kmask = (np.arange(K)[None, :] < tok_len[:, None]).astype(np.float32)

    in_maps = []
    for c in range(N_CORES):
        s = slice(c * BPC, (c + 1) * BPC)
        in_maps.append({
            "encp": enc_pad[s], "encTp": encTp[s], "wT": wT,
            "convb": convb_r, "projw": projw_r, "projb": projb_v,
            "tokc": tokc, "tri_s": tri_strict, "ones2": ones2,
            "tmask": tmask[s], "kmask": kmask[s], "toklen": toklen_f[s],
        })

    nc = _get_program()
    trace = bool(os.environ.get("KERNEL_TRACE"))
    res = run_bass_kernel_spmd(nc, in_maps, list(range(N_CORES)), trace=trace)
    LAST_EXEC_NS = res.exec_time_ns
    LAST_RESULTS = res

    acoustic = np.concatenate([res.results[c]["acoustic"]
                               for c in range(N_CORES)], axis=0)
    raw_mass = np.concatenate([res.results[c]["raw_mass"]
                               for c in range(N_CORES)], axis=0)
    quantity_loss = np.float32(np.mean(np.abs(raw_mass - toklen_f)))
    return acoustic, tok_len, quantity_loss


# revision 12
# speedup vs baseline: 1.1744x; 1.0258x over previous
"""CIF token bridge kernel for Trainium2 (8 NeuronCores, data-parallel).

Per batch (B=16, H=1024, T=2048, K=192):
  conv(k=3, SAME) over time -> silu -> proj -> sigmoid -> masked alpha
  -> raw_mass / rescale -> cumsum centers -> triangular assignment
  -> normalized einsum against raw features.

Sharding: batch dim across 8 cores (2 batches/core); all params replicated.
Matmuls run in float32r (fast fp32 mode, ~11 mantissa bits); the CIF
cumsum runs in true fp32 via a two-level scan (free-dim hardware scan +
a strict-triangular fp32 matmul for row offsets) to keep center drift
~1e-5.

Time layout for the assignment/einsum stages is "p-major": tile column c,
partition p holds t = p*16 + c. The host pre-permutes the feature matrix
(encT) to match, so the einsum contraction sees a consistent t ordering.
"""
import os
import sys
import types

import numpy as np


def _install_ntff_hook():
    """antenv.axon_hooks is absent from this image; inject it and register
    the ctypes NTFF hook so trace=True yields HW exec times."""
    if "antenv.axon_hooks" in sys.modules:
        return
    mod = types.ModuleType("antenv.axon_hooks")
    mod._hook = None
    mod.set_axon_ntff_profile_hook = lambda h: setattr(mod, "_hook", h)
    mod.get_axon_ntff_profile_hook = lambda: mod._hook
    sys.modules["antenv.axon_hooks"] = mod
    try:
        import antenv

        antenv.axon_hooks = mod
    except ImportError:
        pass
    try:
        from trn_agent_boot.trn_boot import _ntff_profile_via_ctypes

        hook = _ntff_profile_via_ctypes("/opt/axon/libaxon_pjrt.so")
        if hook is not None:
            mod._hook = hook
    except Exception:
        pass


_install_ntff_hook()

import concourse.bass as bass  # noqa: E402
import concourse.tile as tile  # noqa: E402
from concourse import bacc, mybir  # noqa: E402
from concourse.bass_utils import run_bass_kernel_spmd  # noqa: E402

N_CORES = 8
B, H, T = 16, 1024, 2048
K = 192
MAX_TOK = 192
BPC = B // N_CORES  # batches per core
NT = T // 512       # 512-wide t tiles per batch
NH = H // 128       # 128-row chunks of H
NC16 = T // 128     # t chunks per batch (tile columns in p-major layout)

f32 = mybir.dt.float32
f32r = mybir.dt.float32r
AF = mybir.ActivationFunctionType
ALU = mybir.AluOpType
AX = mybir.AxisListType

LAST_EXEC_NS = None
LAST_RESULTS = None


def _build_program():
    nc = bacc.Bacc(trn_type="TRN2", target_bir_lowering=False, debug=False,
                   num_devices=1)

    encp = nc.dram_tensor("encp", [BPC, H, T + 2], f32r, kind="ExternalInput").ap()
    # encTp[b, c, p, :] = feats[b, p*16 + c, :]
    encTp = nc.dram_tensor("encTp", [BPC, NC16, 128, H], f32r,
                           kind="ExternalInput").ap()
    wT = nc.dram_tensor("wT", [3, H, H], f32r, kind="ExternalInput").ap()
    convb = nc.dram_tensor("convb", [128, NH], f32, kind="ExternalInput").ap()
    projw = nc.dram_tensor("projw", [128, NH], f32r, kind="ExternalInput").ap()
    projb = nc.dram_tensor("projb", [1, 1], f32, kind="ExternalInput").ap()
    tokc = nc.dram_tensor("tokc", [128, K], f32, kind="ExternalInput").ap()
    # tri_s[p, m] = 1.0 if p < m else 0.0 (strict lower prefix as lhsT)
    tri_s = nc.dram_tensor("tri_s", [128, 128], f32, kind="ExternalInput").ap()
    ones2 = nc.dram_tensor("ones2", [128, 2], f32r, kind="ExternalInput").ap()
    tmask = nc.dram_tensor("tmask", [BPC, T], f32, kind="ExternalInput").ap()
    kmask = nc.dram_tensor("kmask", [BPC, K], f32, kind="ExternalInput").ap()
    toklen = nc.dram_tensor("toklen", [BPC], f32, kind="ExternalInput").ap()

    acoustic = nc.dram_tensor("acoustic", [BPC, K, H], f32,
                              kind="ExternalOutput").ap()
    rm_out = nc.dram_tensor("raw_mass", [BPC], f32, kind="ExternalOutput").ap()

    with tile.TileContext(nc) as tc:
        with (
            tc.tile_pool(name="wp", bufs=24) as wp,
            tc.tile_pool(name="cst", bufs=1) as cst,
            tc.tile_pool(name="encs", bufs=12) as encs,
            tc.tile_pool(name="sp", bufs=3) as sp,
            tc.tile_pool(name="rows", bufs=2) as rows,
            tc.tile_pool(name="pm", bufs=2) as pm,
            tc.tile_pool(name="asgp", bufs=17) as asgp,
            tc.tile_pool(name="etp", bufs=6) as etp,
            tc.tile_pool(name="otp", bufs=2) as otp,
            tc.tile_pool(name="cps", bufs=2, space="PSUM") as cpsp,
            tc.tile_pool(name="pps", bufs=1, space="PSUM") as ppsp,
            tc.tile_pool(name="xps", bufs=1, space="PSUM") as xpsp,
            tc.tile_pool(name="e0ps", bufs=2, space="PSUM") as e0psp,
            tc.tile_pool(name="e1ps", bufs=2, space="PSUM") as e1psp,
        ):
            # ---- small constants first (cheap, unblock early compute) ----
            convb_sb = cst.tile([128, NH], f32)
            nc.sync.dma_start(convb_sb[:], convb[:])
            projw_sb = cst.tile([128, NH], f32r)
            nc.sync.dma_start(projw_sb[:], projw[:])
            projb_sb = cst.tile([1, 1], f32)
            nc.sync.dma_start(projb_sb[:], projb[:])
            tokc_sb = cst.tile([128, K], f32)
            nc.sync.dma_start(tokc_sb[:], tokc[:])
            tri_sb = cst.tile([128, 128], f32)
            nc.sync.dma_start(tri_sb[:], tri_s[:])
            ones_sb = cst.tile([128, 2], f32r)
            nc.sync.dma_start(ones_sb[:], ones2[:])

            # ---- first batch\'s first enc slab BEFORE the 12.6MB of weights
            # so the PE isn\'t starved at kernel start ----
            enc_slabs = {}

            def load_slab(b, t):
                tiles = []
                for i in range(NH):
                    et = encs.tile([128, 514], f32r, tag="enc",
                                   name=f"enc{b}_{t}_{i}")
                    nc.sync.dma_start(
                        et[:], encp[b, i * 128:(i + 1) * 128,
                                    t * 512:t * 512 + 514])
                    tiles.append(et)
                enc_slabs[(b, t)] = tiles

            load_slab(0, 0)

            w_sb = []
            for k in range(3):
                for i in range(NH):
                    wt = wp.tile([128, H], f32r, tag="w", name=f"w{k}_{i}")
                    nc.sync.dma_start(wt[:], wT[k, i * 128:(i + 1) * 128, :])
                    w_sb.append(wt)

            alpha_rows = {}
            rm_parts = {}

            def conv_tile(b, t):
                """conv -> silu -> proj -> sigmoid*mask for one 512 tile."""
                if (b, t) not in enc_slabs:
                    load_slab(b, t)
                enc_t = enc_slabs.pop((b, t))
                if t == 0:
                    alpha_rows[b] = rows.tile([1, T], f32, tag="alpha",
                                              name=f"alpha{b}")
                    rm_parts[b] = pm.tile([1, NT], f32, tag="rm4",
                                          name=f"rm4_{b}")
                alpha_row = alpha_rows[b]
                pps = ppsp.tile([1, 512], f32, tag="proj", name=f"pj{b}_{t}")
                for o in range(NH):
                    cps = cpsp.tile([128, 512], f32, tag="conv",
                                    name=f"cv{b}_{t}_{o}")
                    for k in range(3):
                        for i in range(NH):
                            nc.tensor.matmul(
                                cps[:],
                                w_sb[k * NH + i][:, o * 128:(o + 1) * 128],
                                enc_t[i][:, k:k + 512],
                                start=(k == 0 and i == 0),
                                stop=(k == 2 and i == NH - 1))
                    s_t = sp.tile([128, 512], f32r, tag="s",
                                  name=f"s{b}_{t}_{o}")
                    nc.scalar.activation(s_t[:], cps[:], AF.Silu,
                                         bias=convb_sb[:, o:o + 1], scale=1.0)
                    nc.tensor.matmul(pps[:], projw_sb[:, o:o + 1], s_t[:],
                                     start=(o == 0), stop=(o == NH - 1))
                sig = sp.tile([1, 512], f32, tag="sig", bufs=2,
                              name=f"sig{b}_{t}")
                nc.scalar.activation(sig[:], pps[:], AF.Sigmoid,
                                     bias=projb_sb[0:1, 0:1], scale=1.0)
                tmseg = sp.tile([1, 512], f32, tag="tmseg", bufs=3,
                                name=f"tms{b}_{t}")
                nc.sync.dma_start(tmseg[:], tmask[b, t * 512:(t + 1) * 512])
                nc.vector.scalar_tensor_tensor(
                    alpha_row[0:1, t * 512:(t + 1) * 512], sig[:], 0.0,
                    tmseg[:], op0=ALU.bypass, op1=ALU.mult,
                    accum_out=rm_parts[b][0:1, t:t + 1])

            def phase_b(b):
                """raw mass, rescale, two-level cumsum, centers (p-major)."""
                alpha_row = alpha_rows[b]
                # p-major reshape first (overlaps the scalar chain):
                # a_pm[p, c] = alpha[p*16 + c]
                a_pm = pm.tile([128, NC16], f32, tag="a_pm", name=f"apm{b}")
                nc.sync.dma_start(a_pm[:], alpha_row[:])
                rm = pm.tile([1, 1], f32, tag="rm", name=f"rm{b}")
                nc.vector.tensor_reduce(rm[:], rm_parts[b][:], axis=AX.X,
                                        op=ALU.add)
                nc.sync.dma_start(rm_out[b:b + 1], rm[:])
                rmc = pm.tile([1, 1], f32, tag="rmc", name=f"rmc{b}")
                nc.vector.tensor_scalar_max(rmc[:], rm[:], 1e-6)
                rmi = pm.tile([1, 1], f32, tag="rmi", name=f"rmi{b}")
                nc.vector.reciprocal(rmi[:], rmc[:])
                tl_sb = pm.tile([1, 1], f32, tag="tl", name=f"tl{b}")
                nc.sync.dma_start(tl_sb[:], toklen[b:b + 1])
                scl = pm.tile([1, 1], f32, tag="scl", name=f"scl{b}")
                nc.vector.tensor_mul(scl[:], rmi[:], tl_sb[:])
                # scl lives on partition 0 only; broadcast for [128,*] ops
                sclb = pm.tile([128, 1], f32, tag="sclb", name=f"sclb{b}")
                nc.gpsimd.partition_broadcast(sclb[:], scl[0:1, 0:1])
                sa_pm = pm.tile([128, NC16], f32, tag="sa_pm", name=f"sapm{b}")
                nc.vector.tensor_scalar(sa_pm[:], a_pm[:], sclb[:, 0:1],
                                        None, op0=ALU.mult)
                # in-row inclusive prefix (free-dim hardware scan)
                pref = pm.tile([128, NC16], f32, tag="pref", name=f"pref{b}")
                nc.vector.tensor_tensor_scan(pref[:], sa_pm[:], sa_pm[:],
                                             0.0, op0=ALU.add, op1=ALU.bypass)
                # row offsets: offs[m] = sum_{p<m} rowtot[p] (strict tri fp32)
                offs = xpsp.tile([128, 2], f32, tag="small", name=f"offs{b}")
                nc.tensor.matmul(offs[:, 0:2], tri_sb[:],
                                 pref[:, NC16 - 2:NC16], start=True, stop=True)
                # centers = pref + offs - 0.5*sa ; negated copies for ACT bias
                cmb = pm.tile([128, NC16], f32, tag="cmb", name=f"cmb{b}")
                nc.vector.scalar_tensor_tensor(cmb[:], sa_pm[:], -0.5,
                                               pref[:], op0=ALU.mult,
                                               op1=ALU.add)
                cent_neg = pm.tile([128, NC16], f32, tag="cent_neg",
                                   name=f"cn{b}")
                nc.vector.tensor_scalar(cent_neg[:], cmb[:], offs[:, 1:2],
                                        -1.0, op0=ALU.add, op1=ALU.mult)
                sa_neg = pm.tile([128, NC16], f32, tag="sa_neg", name=f"sn{b}")
                nc.vector.tensor_scalar_mul(sa_neg[:], sa_pm[:], -1.0)
                return sa_pm, cent_neg, sa_neg

            asg_tiles = {}

            def phase_c(b, sa_pm, cent_neg, sa_neg):
                """assignment tiles: relu(1-|tokc-cent|)*sa, f32r."""
                asg = []
                for c in range(NC16):
                    td = asgp.tile([128, K], f32, tag="td", bufs=3,
                                   name=f"td{b}_{c}")
                    nc.scalar.activation(td[:], tokc_sb[:], AF.Abs,
                                         bias=cent_neg[:, c:c + 1], scale=1.0)
                    at = asgp.tile([128, K], f32r, tag="asg",
                                   name=f"at{b}_{c}")
                    nc.scalar.activation(at[:], td[:], AF.Relu,
                                         bias=sa_pm[:, c:c + 1],
                                         scale=sa_neg[:, c:c + 1])
                    asg.append(at)
                asg_tiles[b] = asg

            scs_tiles = {}

            def phase_d_denom(b):
                asg = asg_tiles[b]
                kms = []
                for kc, m in ((0, 128), (1, 64)):
                    km = pm.tile([m, 1], f32, tag=f"km{kc}",
                                 name=f"km{b}_{kc}")
                    nc.sync.dma_start(km[:], kmask[b, kc * 128:kc * 128 + m])
                    kms.append(km)
                scs = []
                for kc, m in ((0, 128), (1, 64)):
                    dn = xpsp.tile([128, 2], f32, tag="small",
                                   name=f"dn{b}_{kc}")
                    for c in range(NC16):
                        nc.tensor.matmul(dn[:m, 0:2],
                                         asg[c][:, kc * 128:kc * 128 + m],
                                         ones_sb[:], start=(c == 0),
                                         stop=(c == NC16 - 1))
                    sc = pm.tile([m, 1], f32, tag=f"sc{kc}",
                                 name=f"sc{b}_{kc}")
                    nc.vector.tensor_scalar_max(sc[:], dn[:m, 0:1], 1e-6)
                    sci = pm.tile([m, 1], f32, tag=f"sci{kc}",
                                  name=f"sci{b}_{kc}")
                    nc.vector.reciprocal(sci[:], sc[:])
                    scm = pm.tile([m, 1], f32, tag=f"scm{kc}",
                                  name=f"scm{b}_{kc}")
                    nc.vector.tensor_mul(scm[:], sci[:], kms[kc][:])
                    scs.append(scm)
                scs_tiles[b] = scs

            ein_psums = {}

            def phase_d_einsum_mm(b, hh):
                asg = asg_tiles[b]
                ps0 = e0psp.tile([128, 512], f32, tag="e0",
                                 name=f"e0{b}_{hh}")
                ps1 = e1psp.tile([64, 512], f32, tag="e1", name=f"e1{b}_{hh}")
                for c in range(NC16):
                    et = etp.tile([128, 512], f32r, tag="et",
                                  name=f"et{b}_{hh}_{c}")
                    nc.sync.dma_start(
                        et[:], encTp[b, c, :, hh * 512:(hh + 1) * 512])
                    nc.tensor.matmul(ps0[:], asg[c][:, 0:128], et[:],
                                     start=(c == 0), stop=(c == NC16 - 1))
                    nc.tensor.matmul(ps1[:], asg[c][:, 128:192], et[:],
                                     start=(c == 0), stop=(c == NC16 - 1))
                ein_psums[(b, hh)] = [ps0, ps1]

            def phase_d_out(b, hh):
                scs = scs_tiles[b]
                pss = ein_psums.pop((b, hh))
                for kc, m in ((0, 128), (1, 64)):
                    ot = otp.tile([m, 512], f32, tag=f"ot{kc}",
                                  name=f"ot{b}_{hh}_{kc}")
                    nc.vector.tensor_scalar(ot[:], pss[kc][:],
                                            scs[kc][:, 0:1], None,
                                            op0=ALU.mult)
                    nc.sync.dma_start(
                        acoustic[b, kc * 128:kc * 128 + m,
                                 hh * 512:(hh + 1) * 512], ot[:])

            # ---- emission schedule ----
            for t in range(NT):
                conv_tile(0, t)
            sa0 = phase_b(0)
            phase_c(0, *sa0)
            # interleave einsum(b0) with conv(b1) to spread encT DMA load
            conv_tile(1, 0)
            phase_d_einsum_mm(0, 0)
            conv_tile(1, 1)
            phase_d_einsum_mm(0, 1)
            conv_tile(1, 2)
            phase_d_denom(0)
            phase_d_out(0, 0)
            phase_d_out(0, 1)
            conv_tile(1, 3)
            sa1 = phase_b(1)
            phase_c(1, *sa1)
            phase_d_einsum_mm(1, 0)
            phase_d_einsum_mm(1, 1)
            phase_d_denom(1)
            phase_d_out(1, 0)
            phase_d_out(1, 1)
    nc.compile()
    return nc


_PROGRAM = None


def _get_program():
    global _PROGRAM
    if _PROGRAM is None:
        _PROGRAM = _build_program()
    return _PROGRAM


def kernel(encoder_outputs, encoder_lengths, target_token_lengths,
           conv_w, conv_b, proj_w, proj_b):
    global LAST_EXEC_NS, LAST_RESULTS
    enc = np.asarray(encoder_outputs, dtype=np.float32)
    enc_len = np.asarray(encoder_lengths, dtype=np.int32)
    tgt_len = np.asarray(target_token_lengths, dtype=np.int32)
    conv_w = np.asarray(conv_w, dtype=np.float32)
    conv_b = np.asarray(conv_b, dtype=np.float32)
    proj_w = np.asarray(proj_w, dtype=np.float32)
    proj_b = np.asarray(proj_b, dtype=np.float32)

    enc_pad = np.zeros((B, H, T + 2), dtype=np.float32)
    enc_pad[:, :, 1:-1] = enc
    # p-major permuted feats: encTp[b, c, p, :] = enc[b, :, p*16 + c].T
    encTp = np.ascontiguousarray(
        enc.transpose(0, 2, 1).reshape(B, 128, NC16, H).transpose(0, 2, 1, 3))
    wT = np.ascontiguousarray(conv_w.transpose(2, 1, 0))
    convb_r = np.ascontiguousarray(conv_b.reshape(NH, 128).T)
    projw_r = np.ascontiguousarray(proj_w.reshape(NH, 128).T)
    projb_v = proj_b.reshape(1, 1)
    tokc = np.ascontiguousarray(
        np.broadcast_to(np.arange(K, dtype=np.float32) + 0.5, (128, K)))
    tri_strict = np.triu(np.ones((128, 128), dtype=np.float32), 1)
    ones2 = np.ones((128, 2), dtype=np.float32)

    tok_len = np.clip(tgt_len, 1, MAX_TOK).astype(np.int32)
    toklen_f = tok_len.astype(np.float32)
    tmask = (np.arange(T)[None, :] < enc_len[:, None]).astype(np.float32)
    kmask = (np.arange(K)[None, :] < tok_len[:, None]).astype(np.float32)

    in_maps = []
    for c in range(N_CORES):
        s = slice(c * BPC, (c + 1) * BPC)
        in_maps.append({
            "encp": enc_pad[s], "encTp": encTp[s], "wT": wT,
            "convb": convb_r, "projw": projw_r, "projb": projb_v,
            "tokc": tokc, "tri_s": tri_strict, "ones2": ones2,
            "tmask": tmask[s], "kmask": kmask[s], "toklen": toklen_f[s],
        })

    nc = _get_program()
    trace = bool(os.environ.get("KERNEL_TRACE"))
    res = run_bass_kernel_spmd(nc, in_maps, list(range(N_CORES)), trace=trace)
    LAST_EXEC_NS = res.exec_time_ns
    LAST_RESULTS = res

    acoustic = np.concatenate([res.results[c]["acoustic"]
                               for c in range(N_CORES)], axis=0)
    raw_mass = np.concatenate([res.results[c]["raw_mass"]
                               for c in range(N_CORES)], axis=0)
    quantity_loss = np.float32(np.mean(np.abs(raw_mass - toklen_f)))
    return acoustic, tok_len, quantity_loss
